# revision 9
# baseline (speedup 1.0000x reference)
"""Trainium2 Bass kernel for the inverse deep-hough-transform gather-reduce.

out[n, c, y, x] = sum_k acc[n, c, k, rho_idx[k, y, x]]

Design (v4): one-hot matmul gather on the PE (tensor engine)
------------------------------------------------------------
For a fixed output row y and angle k, the gather over x is a selection
matmul:  out[x, nc] += sum_rho OH[rho, x] * acc_k[rho, nc], with OH the
0/1 one-hot of rho == r(k, y, x).  The PE streams the 512 nc columns at
1 col/cycle and produces >= 128 gathered elements per cycle.

- Contraction dim K packs multiple angles' rho *windows* (bin packing):
  angle k needs a window of win_g(k) rho rows covering a y-block of g(k)
  rows (g in {16,8,4,2} per angle; finer g for |cos| ~ 1 angles whose
  window drifts fast with y).  First-fit-decreasing packs the windows
  into 128-row bins; one bin = one matmul per y, summing all its angles.
- Sharding: bins are dealt across the 8 cores class-by-class so the SPMD
  instruction stream is identical on every core; all per-core geometry
  lives in host-built data (one-hot weight tiles + rho window "slabs").
  Host sums the 8 per-core partial outputs.
- Per y: P (~18) accumulating matmuls into one PSUM bank (8 banks
  cycle), ACT evicts PSUM->SBUF, sync DMAs the row out to HBM.  Weight
  tiles and slab blocks stream HBM->SBUF on the gpsimd queue.
- Sync uses one semaphore per SBUF slot (weight-ring slot / slab
  double-buffer half) so correctness does not depend on cross-DMA
  completion ordering: successive DMAs into the *same* slot are already
  serialized by the consumption flow control.
"""

from contextlib import ExitStack

import ml_dtypes
import numpy as np

import concourse.bass as bass
from concourse import mybir
from concourse.bass_utils import run_bass_kernel_spmd

BF16 = ml_dtypes.bfloat16
FP8 = ml_dtypes.float8_e4m3

# Problem constants (hardcoded per the harness contract).
N, C, A, R = 4, 128, 180, 184
H = W = 128
NC = N * C  # 512
NCORES = 8
NY = H  # output rows, one PSUM accumulation group each
NBANK = 8  # PSUM banks
NWRING = 8  # weight ring depth (y slots)
NOBUF = 4  # output staging buffers

_cache = {}


def _rho_table():
    """r[k, y, x] int32 rho index; always in [0, R) for this geometry."""
    if "r" not in _cache:
        k = np.arange(A)
        theta = k * (np.pi / A)
        cos_t, sin_t = np.cos(theta), np.sin(theta)
        y, x = np.meshgrid(np.arange(H), np.arange(W), indexing="ij")
        xc = (x - W // 2).astype(np.float64)
        yc = (y - H // 2).astype(np.float64)
        r = np.round(cos_t[:, None, None] * xc[None] + sin_t[:, None, None] * yc[None])
        r = r.astype(np.int64) + R // 2
        assert (r >= 0).all() and (r < R).all()
        _cache["r"] = r.astype(np.int32)
    return _cache["r"]


def _geometry():
    """Static geometry: per-core bin plan + DMA schedule (instruction
    stream identical across cores; only data differs)."""
    if "geo" in _cache:
        return _cache["geo"]
    r = _rho_table()
    lo = r.min(axis=2)  # [A, H]
    hi = r.max(axis=2)

    def win_at_g(k, g):
        w = 0
        for b in range(0, NY, g):
            w = max(w, int(hi[k, b : b + g].max() - lo[k, b : b + g].min()) + 1)
        return w

    gk = {}
    for k in range(A):
        for g in (16, 8, 4, 2):
            if win_at_g(k, g) <= 128:
                gk[k] = g
                break
        assert k in gk

    # FFD bin packing per granularity class.
    def ffd(items):
        bins = []
        for w, k in sorted(items, reverse=True):
            for b in bins:
                if b[0] + w <= 128:
                    b[0] += w
                    b[1].append((k, w))
                    break
            else:
                bins.append([w, [(k, w)]])
        return [b[1] for b in bins]

    # Pack each granularity class, then deal ALL bins sorted finest-g
    # first into groups of 8 (one per core); a position's refresh rate is
    # the finest g in its group (finer refresh of a coarser lane is
    # always valid - the window only shrinks).
    all_bins = []  # (g, lanes)
    for g in (2, 4, 8, 16):
        items = [(win_at_g(k, g), k) for k in range(A) if gk[k] == g]
        all_bins += [(g, b) for b in ffd(items)]
    while len(all_bins) % NCORES:
        all_bins.append((16, []))
    P = len(all_bins) // NCORES
    profile = [min(g for g, _ in all_bins[j * NCORES : (j + 1) * NCORES])
               for j in range(P)]

    # lanes[c][i] = list of (k, width, base_row); bases are prefix sums.
    lanes = [[] for _ in range(NCORES)]
    for c in range(NCORES):
        for j in range(P):
            _, lane_list = all_bins[j * NCORES + c]
            out, base = [], 0
            for k, w in lane_list:
                out.append((k, w, base))
                base += w
            assert base <= 128
            lanes[c].append(out)

    # Slab slots: position i has NY // g_i blocks, double buffered.
    slot_of = {}
    nslot = 0
    for i, g in enumerate(profile):
        for b in range(NY // g):
            slot_of[(i, b)] = nslot
            nslot += 1

    # DMA schedule sorted by issue key: (key, tie, kind, i, b, flow_wait).
    # Slab blocks are placed half a weight-ring early in the stream so the
    # block-boundary bursts are not stuck behind weight chunks whose flow
    # waits release later; their own flow waits keep it correct.
    items = []
    for i, g in enumerate(profile):
        for b in range(NY // g):
            fw = (b - 1) * g if b >= 2 else None
            # key <= deadline (b*g) and key > fw: every item the flow wait
            # depends on sorts earlier -> deadlock-free.
            key = max(0 if fw is None else fw + 1, b * g - NWRING // 2)
            items.append((key, 0, "slab", i, b, fw))
    for y in range(NY):
        fw = y - NWRING + 1 if y >= NWRING else None
        items.append((y, 1, "wt", y, 0, fw))
    items.sort(key=lambda t: (t[0], t[1]))

    def lane_off(k, width, g, b):
        l = int(lo[k, b * g : (b + 1) * g].min())
        h = int(hi[k, b * g : (b + 1) * g].max())
        assert h - l + 1 <= width
        return min(l, R - width)

    _cache["geo"] = dict(
        profile=profile, P=P, lanes=lanes, slot_of=slot_of, nslot=nslot,
        items=items, lane_off=lane_off,
    )
    return _cache["geo"]


def _host_tables():
    """Per-core one-hot weight tables (geometry only; cached across calls)
    and slab assembly metadata."""
    if "wts" in _cache:
        return _cache["wts"], _cache["slab_meta"]
    geo = _geometry()
    r = _rho_table()
    P = geo["P"]
    profile = geo["profile"]
    wts = []
    slab_meta = []  # per core: list over slots of [(k, o, width, base)]
    xs = np.arange(W)
    ys = np.arange(NY)
    for c in range(NCORES):
        w = np.zeros((NY, 128, P * 128), BF16)
        meta = [[] for _ in range(geo["nslot"])]
        for i, g in enumerate(profile):
            for k, width, base in geo["lanes"][c][i]:
                for b in range(NY // g):
                    o = geo["lane_off"](k, width, g, b)
                    meta[geo["slot_of"][(i, b)]].append((k, o, width, base))
                    yb = ys[b * g : (b + 1) * g]
                    rowidx = r[k, yb] - o + base  # [g, W]
                    w[yb[:, None], rowidx, i * 128 + xs[None, :]] = 1
        wts.append(w)
        slab_meta.append(meta)
    _cache["wts"] = wts
    _cache["slab_meta"] = slab_meta
    return wts, slab_meta


def _build_nc():
    if "nc" in _cache:
        return _cache["nc"]
    geo = _geometry()
    P = geo["P"]
    profile = geo["profile"]
    nslot = geo["nslot"]

    nc = bass.Bass("TRN2", debug=False, target_bir_lowering=False, num_devices=NCORES)
    wts_d = nc.dram_tensor(
        "wts", [NY, 128, P * 128], mybir.dt.bfloat16, kind="ExternalInput"
    ).ap()
    slab_d = nc.dram_tensor(
        "slabs", [nslot, 128, NC], mybir.dt.bfloat16, kind="ExternalInput"
    ).ap()
    out_d = nc.dram_tensor(
        "out", [NY, 128, NC], mybir.dt.float32, kind="ExternalOutput"
    ).ap()

    ctx = ExitStack()
    _cache["ctx"] = ctx
    slabs_sb = ctx.enter_context(
        nc.sbuf_tensor("slabs_sb", [128, P * 2 * NC], mybir.dt.bfloat16)
    )
    wring = ctx.enter_context(
        nc.sbuf_tensor("wring", [128, NWRING * P * 128], mybir.dt.bfloat16)
    )
    obuf = ctx.enter_context(
        nc.sbuf_tensor("obuf", [128, NOBUF * NC], mybir.dt.float32)
    )
    ps = [
        ctx.enter_context(nc.psum_tensor(f"ps{i}", [128, NC], mybir.dt.float32))
        for i in range(NBANK)
    ]
    mm_sem = ctx.enter_context(nc.semaphore("mm_sem"))
    cp_sem = ctx.enter_context(nc.semaphore("cp_sem"))
    dump_sem = ctx.enter_context(nc.semaphore("dump_sem"))
    wt_sems = [
        ctx.enter_context(nc.semaphore(f"wt{s}")) for s in range(NWRING)
    ]
    sl_sems = [
        [ctx.enter_context(nc.semaphore(f"sl{i}_{h}")) for h in range(2)]
        for i in range(P)
    ]
    block = ctx.enter_context(nc.Block())

    def slab_col(i, half):
        return (i * 2 + half) * NC

    @block.gpsimd
    def _(gpsimd):
        for _, _, kind, i, b, fw in geo["items"]:
            if fw is not None:
                gpsimd.wait_ge(mm_sem, fw)
            if kind == "slab":
                col = slab_col(i, b % 2)
                gpsimd.dma_start(
                    slabs_sb[:, col : col + NC], slab_d[geo["slot_of"][(i, b)]]
                ).then_inc(sl_sems[i][b % 2], 16)
            else:
                y = i
                base = (y % NWRING) * P * 128
                gpsimd.dma_start(
                    wring[:, base : base + P * 128], wts_d[y]
                ).then_inc(wt_sems[y % NWRING], 16)

    @block.tensor
    def _(tensor):
        for y in range(NY):
            if y >= NBANK:
                tensor.wait_ge(cp_sem, y - NBANK + 1)
            tensor.wait_ge(wt_sems[y % NWRING], 16 * (y // NWRING + 1))
            for i, g in enumerate(profile):
                if y % g == 0:
                    b = y // g
                    tensor.wait_ge(sl_sems[i][b % 2], 16 * (b // 2 + 1))
            wbase = (y % NWRING) * P * 128
            for i, g in enumerate(profile):
                col = slab_col(i, (y // g) % 2)
                mm = tensor.matmul(
                    out=ps[y % NBANK][:],
                    lhsT=wring[:, wbase + i * 128 : wbase + (i + 1) * 128],
                    rhs=slabs_sb[:, col : col + NC],
                    start=(i == 0),
                    stop=(i == P - 1),
                )
            mm.then_inc(mm_sem, 1)

    @block.scalar
    def _(scalar):
        for y in range(NY):
            scalar.wait_ge(mm_sem, y + 1)
            if y >= NOBUF:
                scalar.wait_ge(dump_sem, 16 * (y - NOBUF + 1))
            col = (y % NOBUF) * NC
            scalar.copy(obuf[:, col : col + NC], ps[y % NBANK][:]).then_inc(cp_sem, 1)

    @block.sync
    def _(sync):
        for y in range(NY):
            sync.wait_ge(cp_sem, y + 1)
            col = (y % NOBUF) * NC
            sync.dma_start(out_d[y], obuf[:, col : col + NC]).then_inc(dump_sem, 16)

    _cache["nc"] = nc
    return nc


def _install_ntff_hook():
    """Provide the antenv.axon_hooks shim the image lacks, wiring the
    ctypes NTFF profiler from trn_agent_boot."""
    import sys
    import types

    if "antenv.axon_hooks" in sys.modules:
        return
    import antenv
    from trn_agent_boot.trn_boot import _ntff_profile_via_ctypes

    mod = types.ModuleType("antenv.axon_hooks")
    hook = _ntff_profile_via_ctypes("/opt/axon/libaxon_pjrt.so")
    mod.get_axon_ntff_profile_hook = lambda: hook
    mod.set_axon_ntff_profile_hook = lambda h: None
    sys.modules["antenv.axon_hooks"] = mod
    antenv.axon_hooks = mod


def hw_exec_time_ns(trace_cores=None):
    """Re-run the last kernel() invocation with tracing; return max core ns."""
    _install_ntff_hook()
    nc = _cache["nc"]
    res = run_bass_kernel_spmd(
        nc,
        _cache["in_maps"],
        core_ids=list(range(NCORES)),
        trace=True,
        trace_cores=trace_cores,
    )
    _cache["trace"] = res
    return res.exec_time_ns


def kernel(accumulator, out_H=128, out_W=128, numangle=180, numrho=184):
    accumulator = np.asarray(accumulator, np.float32)
    assert accumulator.shape == (N, C, A, R), accumulator.shape
    assert int(out_H) == H and int(out_W) == W
    assert int(numangle) == A and int(numrho) == R

    geo = _geometry()
    wts, slab_meta = _host_tables()
    nc = _build_nc()

    # acc_t[k, rho, nc] bf16 - slab source.
    acc_t = np.ascontiguousarray(
        accumulator.reshape(NC, A, R).transpose(1, 2, 0)
    ).astype(BF16)

    in_maps = []
    for c in range(NCORES):
        slabs = np.zeros((geo["nslot"], 128, NC), BF16)
        for slot, entries in enumerate(slab_meta[c]):
            for k, o, width, base in entries:
                slabs[slot, base : base + width] = acc_t[k, o : o + width]
        in_maps.append({"wts": wts[c], "slabs": slabs})
    _cache["in_maps"] = in_maps
    res = run_bass_kernel_spmd(nc, in_maps, core_ids=list(range(NCORES)))

    # Unshard: sum the 8 per-core partials.  out[y, x, nc]
    total = np.zeros((NY, 128, NC), np.float64)
    for c in range(NCORES):
        total += res.results[c]["out"]
    return (
        total.transpose(2, 0, 1).reshape(N, C, H, W).astype(np.float32)
    )


# revision 12
# speedup vs baseline: 1.7727x; 1.7727x over previous
"""Trainium2 Bass kernel for the inverse deep-hough-transform gather-reduce.

out[n, c, y, x] = sum_k acc[n, c, k, rho_idx[k, y, x]]

Design (v4): one-hot matmul gather on the PE (tensor engine)
------------------------------------------------------------
For a fixed output row y and angle k, the gather over x is a selection
matmul:  out[x, nc] += sum_rho OH[rho, x] * acc_k[rho, nc], with OH the
0/1 one-hot of rho == r(k, y, x).  The PE streams the 512 nc columns at
1 col/cycle and produces >= 128 gathered elements per cycle.

- Contraction dim K packs multiple angles' rho *windows* (bin packing):
  angle k needs a window of win_g(k) rho rows covering a y-block of g(k)
  rows (g in {16,8,4,2} per angle; finer g for |cos| ~ 1 angles whose
  window drifts fast with y).  First-fit-decreasing packs the windows
  into 128-row bins; one bin = one matmul per y, summing all its angles.
- Sharding: bins are dealt across the 8 cores class-by-class so the SPMD
  instruction stream is identical on every core; all per-core geometry
  lives in host-built data (one-hot weight tiles + rho window "slabs").
  Host sums the 8 per-core partial outputs.
- Per y: P (~18) accumulating matmuls into one PSUM bank (8 banks
  cycle), ACT evicts PSUM->SBUF, sync DMAs the row out to HBM.  Weight
  tiles and slab blocks stream HBM->SBUF on the gpsimd queue.
- Sync uses one semaphore per SBUF slot (weight-ring slot / slab
  double-buffer half) so correctness does not depend on cross-DMA
  completion ordering: successive DMAs into the *same* slot are already
  serialized by the consumption flow control.
"""

from contextlib import ExitStack

import ml_dtypes
import numpy as np

import concourse.bass as bass
from concourse import mybir
from concourse.bass_utils import run_bass_kernel_spmd

BF16 = ml_dtypes.bfloat16
FP8 = ml_dtypes.float8_e4m3

# Problem constants (hardcoded per the harness contract).
N, C, A, R = 4, 128, 180, 184
H = W = 128
NC = N * C  # 512
NCORES = 8
NY = H  # output rows, one PSUM accumulation group each
NBANK = 8  # PSUM banks
NWRING = 12  # weight ring depth (y slots)
NOBUF = 8  # output staging buffers

_cache = {}


def _rho_table():
    """r[k, y, x] int32 rho index; always in [0, R) for this geometry."""
    if "r" not in _cache:
        k = np.arange(A)
        theta = k * (np.pi / A)
        cos_t, sin_t = np.cos(theta), np.sin(theta)
        y, x = np.meshgrid(np.arange(H), np.arange(W), indexing="ij")
        xc = (x - W // 2).astype(np.float64)
        yc = (y - H // 2).astype(np.float64)
        r = np.round(cos_t[:, None, None] * xc[None] + sin_t[:, None, None] * yc[None])
        r = r.astype(np.int64) + R // 2
        assert (r >= 0).all() and (r < R).all()
        _cache["r"] = r.astype(np.int32)
    return _cache["r"]


def _geometry():
    """Static geometry: per-core bin plan + DMA schedule (instruction
    stream identical across cores; only data differs)."""
    if "geo" in _cache:
        return _cache["geo"]
    r = _rho_table()
    lo = r.min(axis=2)  # [A, H]
    hi = r.max(axis=2)

    def win_at_g(k, g):
        w = 0
        for b in range(0, NY, g):
            w = max(w, int(hi[k, b : b + g].max() - lo[k, b : b + g].min()) + 1)
        return w

    gk = {}
    for k in range(A):
        for g in (16, 8, 4, 2):
            if win_at_g(k, g) <= 128:
                gk[k] = g
                break
        assert k in gk

    # FFD bin packing per granularity class.
    def ffd(items):
        bins = []
        for w, k in sorted(items, reverse=True):
            for b in bins:
                if b[0] + w <= 128:
                    b[0] += w
                    b[1].append((k, w))
                    break
            else:
                bins.append([w, [(k, w)]])
        return [b[1] for b in bins]

    # Pack each granularity class, then deal ALL bins sorted finest-g
    # first into groups of 8 (one per core); a position's refresh rate is
    # the finest g in its group (finer refresh of a coarser lane is
    # always valid - the window only shrinks).  Positions are then ordered
    # coarse-g first so fine-g slab waits land late in each y's MM group.
    all_bins = []  # (g, lanes)
    for g in (2, 4, 8, 16):
        items = [(win_at_g(k, g), k) for k in range(A) if gk[k] == g]
        all_bins += [(g, b) for b in ffd(items)]
    while len(all_bins) % NCORES:
        all_bins.append((16, []))
    P = len(all_bins) // NCORES
    groups = sorted(
        (all_bins[j * NCORES : (j + 1) * NCORES] for j in range(P)),
        key=lambda grp: -min(g for g, _ in grp),
    )
    profile = [min(g for g, _ in grp) for grp in groups]
    # Slab buffers per position: deep rings for fine-g positions so their
    # frequent refresh gates release far ahead of consumption.
    nbuf = [4 if g >= 16 else 8 for g in profile]

    # lanes[c][i] = list of (k, width, base_row); bases are prefix sums.
    lanes = [[] for _ in range(NCORES)]
    for c in range(NCORES):
        for grp in groups:
            _, lane_list = grp[c]
            out, base = [], 0
            for k, w in lane_list:
                out.append((k, w, base))
                base += w
            assert base <= 128
            lanes[c].append(out)

    # Slab slots: position i has NY // g_i blocks.
    slot_of = {}
    nslot = 0
    for i, g in enumerate(profile):
        for b in range(NY // g):
            slot_of[(i, b)] = nslot
            nslot += 1

    # DMA schedule sorted by issue key: (key, tie, kind, i, b, flow_wait).
    # Slab blocks are placed early in the stream so block-boundary bursts
    # are not stuck behind weight chunks whose flow waits release later.
    items = []
    for i, g in enumerate(profile):
        for b in range(NY // g):
            fw = (b - nbuf[i] + 1) * g if b >= nbuf[i] else None
            # key <= deadline (b*g) and key > fw: every item the flow wait
            # depends on sorts earlier -> deadlock-free.
            key = max(0 if fw is None else fw + 1, b * g - NWRING // 2)
            items.append((key, 0, "slab", i, b, fw))
    for y in range(NY):
        fw = y - NWRING + 1 if y >= NWRING else None
        items.append((y, 1, "wt", y, 0, fw))
    items.sort(key=lambda t: (t[0], t[1]))

    def lane_off(k, width, g, b):
        l = int(lo[k, b * g : (b + 1) * g].min())
        h = int(hi[k, b * g : (b + 1) * g].max())
        assert h - l + 1 <= width
        return min(l, R - width)

    _cache["geo"] = dict(
        profile=profile, P=P, lanes=lanes, slot_of=slot_of, nslot=nslot,
        items=items, lane_off=lane_off, nbuf=nbuf,
        slab_base=np.concatenate([[0], np.cumsum(nbuf)]).tolist(),
    )
    return _cache["geo"]


def _host_tables():
    """Per-core one-hot weight tables (geometry only; cached across calls)
    and slab assembly metadata."""
    if "wts" in _cache:
        return _cache["wts"], _cache["slab_meta"]
    geo = _geometry()
    r = _rho_table()
    P = geo["P"]
    profile = geo["profile"]
    wts = []
    slab_meta = []  # per core: list over slots of [(k, o, width, base)]
    xs = np.arange(W)
    ys = np.arange(NY)
    for c in range(NCORES):
        w = np.zeros((NY, 128, P * 128), BF16)
        meta = [[] for _ in range(geo["nslot"])]
        for i, g in enumerate(profile):
            for k, width, base in geo["lanes"][c][i]:
                for b in range(NY // g):
                    o = geo["lane_off"](k, width, g, b)
                    meta[geo["slot_of"][(i, b)]].append((k, o, width, base))
                    yb = ys[b * g : (b + 1) * g]
                    rowidx = r[k, yb] - o + base  # [g, W]
                    w[yb[:, None], rowidx, i * 128 + xs[None, :]] = 1
        wts.append(w)
        slab_meta.append(meta)
    _cache["wts"] = wts
    _cache["slab_meta"] = slab_meta
    return wts, slab_meta


def _build_nc():
    if "nc" in _cache:
        return _cache["nc"]
    geo = _geometry()
    P = geo["P"]
    profile = geo["profile"]
    nslot = geo["nslot"]

    nc = bass.Bass("TRN2", debug=False, target_bir_lowering=False, num_devices=NCORES)
    wts_d = nc.dram_tensor(
        "wts", [NY, 128, P * 128], mybir.dt.bfloat16, kind="ExternalInput"
    ).ap()
    slab_d = nc.dram_tensor(
        "slabs", [nslot, 128, NC], mybir.dt.bfloat16, kind="ExternalInput"
    ).ap()
    out_d = nc.dram_tensor(
        "out", [NY, 128, NC], mybir.dt.float32, kind="ExternalOutput"
    ).ap()

    ctx = ExitStack()
    _cache["ctx"] = ctx
    SLABCOLS = geo["slab_base"][P] * NC
    slabs_sb = ctx.enter_context(
        nc.sbuf_tensor("slabs_sb", [128, SLABCOLS], mybir.dt.bfloat16)
    )
    wring = ctx.enter_context(
        nc.sbuf_tensor("wring", [128, NWRING * P * 128], mybir.dt.bfloat16)
    )
    obuf = ctx.enter_context(
        nc.sbuf_tensor("obuf", [128, NOBUF * NC], mybir.dt.float32)
    )
    ps = [
        ctx.enter_context(nc.psum_tensor(f"ps{i}", [128, NC], mybir.dt.float32))
        for i in range(NBANK)
    ]
    mm_sem = ctx.enter_context(nc.semaphore("mm_sem"))
    cp_sem = ctx.enter_context(nc.semaphore("cp_sem"))
    dump_sem = ctx.enter_context(nc.semaphore("dump_sem"))
    wt_sems = [
        ctx.enter_context(nc.semaphore(f"wt{s}")) for s in range(NWRING)
    ]
    sl_sems = [
        [ctx.enter_context(nc.semaphore(f"sl{i}_{h}")) for h in range(geo["nbuf"][i])]
        for i in range(P)
    ]
    block = ctx.enter_context(nc.Block())

    def slab_col(i, buf):
        return (geo["slab_base"][i] + buf) * NC

    @block.gpsimd
    def _(gpsimd):
        for _, _, kind, i, b, fw in geo["items"]:
            if fw is not None:
                gpsimd.wait_ge(mm_sem, fw)
            if kind == "slab":
                nb = geo["nbuf"][i]
                col = slab_col(i, b % nb)
                gpsimd.dma_start(
                    slabs_sb[:, col : col + NC], slab_d[geo["slot_of"][(i, b)]]
                ).then_inc(sl_sems[i][b % nb], 16)
            else:
                y = i
                base = (y % NWRING) * P * 128
                gpsimd.dma_start(
                    wring[:, base : base + P * 128], wts_d[y]
                ).then_inc(wt_sems[y % NWRING], 16)

    @block.tensor
    def _(tensor):
        for y in range(NY):
            if y >= NBANK:
                tensor.wait_ge(cp_sem, y - NBANK + 1)
            tensor.wait_ge(wt_sems[y % NWRING], 16 * (y // NWRING + 1))
            wbase = (y % NWRING) * P * 128
            for i, g in enumerate(profile):
                nb = geo["nbuf"][i]
                b = y // g
                if y % g == 0:
                    tensor.wait_ge(sl_sems[i][b % nb], 16 * (b // nb + 1))
                col = slab_col(i, b % nb)
                mm = tensor.matmul(
                    out=ps[y % NBANK][:],
                    lhsT=wring[:, wbase + i * 128 : wbase + (i + 1) * 128],
                    rhs=slabs_sb[:, col : col + NC],
                    start=(i == 0),
                    stop=(i == P - 1),
                )
            mm.then_inc(mm_sem, 1)

    @block.scalar
    def _(scalar):
        for y in range(NY):
            scalar.wait_ge(mm_sem, y + 1)
            if y >= NOBUF:
                scalar.wait_ge(dump_sem, 16 * (y - NOBUF + 1))
            col = (y % NOBUF) * NC
            scalar.copy(obuf[:, col : col + NC], ps[y % NBANK][:]).then_inc(cp_sem, 1)

    @block.sync
    def _(sync):
        for y in range(NY):
            sync.wait_ge(cp_sem, y + 1)
            col = (y % NOBUF) * NC
            sync.dma_start(out_d[y], obuf[:, col : col + NC]).then_inc(dump_sem, 16)

    _cache["nc"] = nc
    return nc


def _install_ntff_hook():
    """Provide the antenv.axon_hooks shim the image lacks, wiring the
    ctypes NTFF profiler from trn_agent_boot."""
    import sys
    import types

    if "antenv.axon_hooks" in sys.modules:
        return
    import antenv
    from trn_agent_boot.trn_boot import _ntff_profile_via_ctypes

    mod = types.ModuleType("antenv.axon_hooks")
    hook = _ntff_profile_via_ctypes("/opt/axon/libaxon_pjrt.so")
    mod.get_axon_ntff_profile_hook = lambda: hook
    mod.set_axon_ntff_profile_hook = lambda h: None
    sys.modules["antenv.axon_hooks"] = mod
    antenv.axon_hooks = mod


def hw_exec_time_ns(trace_cores=None):
    """Re-run the last kernel() invocation with tracing; return max core ns."""
    _install_ntff_hook()
    nc = _cache["nc"]
    res = run_bass_kernel_spmd(
        nc,
        _cache["in_maps"],
        core_ids=list(range(NCORES)),
        trace=True,
        trace_cores=trace_cores,
    )
    _cache["trace"] = res
    return res.exec_time_ns


def kernel(accumulator, out_H=128, out_W=128, numangle=180, numrho=184):
    accumulator = np.asarray(accumulator, np.float32)
    assert accumulator.shape == (N, C, A, R), accumulator.shape
    assert int(out_H) == H and int(out_W) == W
    assert int(numangle) == A and int(numrho) == R

    geo = _geometry()
    wts, slab_meta = _host_tables()
    nc = _build_nc()

    # acc_t[k, rho, nc] bf16 - slab source.
    acc_t = np.ascontiguousarray(
        accumulator.reshape(NC, A, R).transpose(1, 2, 0)
    ).astype(BF16)

    in_maps = []
    for c in range(NCORES):
        slabs = np.zeros((geo["nslot"], 128, NC), BF16)
        for slot, entries in enumerate(slab_meta[c]):
            for k, o, width, base in entries:
                slabs[slot, base : base + width] = acc_t[k, o : o + width]
        in_maps.append({"wts": wts[c], "slabs": slabs})
    _cache["in_maps"] = in_maps
    res = run_bass_kernel_spmd(nc, in_maps, core_ids=list(range(NCORES)))

    # Unshard: sum the 8 per-core partials.  out[y, x, nc]
    total = np.zeros((NY, 128, NC), np.float64)
    for c in range(NCORES):
        total += res.results[c]["out"]
    return (
        total.transpose(2, 0, 1).reshape(N, C, H, W).astype(np.float32)
    )


# revision 17
# speedup vs baseline: 1.7831x; 1.0058x over previous
"""Trainium2 Bass kernel for the inverse deep-hough-transform gather-reduce.

out[n, c, y, x] = sum_k acc[n, c, k, rho_idx[k, y, x]]

Design (v4): one-hot matmul gather on the PE (tensor engine)
------------------------------------------------------------
For a fixed output row y and angle k, the gather over x is a selection
matmul:  out[x, nc] += sum_rho OH[rho, x] * acc_k[rho, nc], with OH the
0/1 one-hot of rho == r(k, y, x).  The PE streams the 512 nc columns at
1 col/cycle and produces >= 128 gathered elements per cycle.

- Contraction dim K packs multiple angles' rho *windows* (bin packing):
  angle k needs a window of win_g(k) rho rows covering a y-block of g(k)
  rows (g in {16,8,4,2} per angle; finer g for |cos| ~ 1 angles whose
  window drifts fast with y).  First-fit-decreasing packs the windows
  into 128-row bins; one bin = one matmul per y, summing all its angles.
- Sharding: bins are dealt across the 8 cores class-by-class so the SPMD
  instruction stream is identical on every core; all per-core geometry
  lives in host-built data (one-hot weight tiles + rho window "slabs").
  Host sums the 8 per-core partial outputs.
- Per y: P (~18) accumulating matmuls into one PSUM bank (8 banks
  cycle), ACT evicts PSUM->SBUF, sync DMAs the row out to HBM.  Weight
  tiles and slab blocks stream HBM->SBUF on the gpsimd queue.
- Sync uses one semaphore per SBUF slot (weight-ring slot / slab
  double-buffer half) so correctness does not depend on cross-DMA
  completion ordering: successive DMAs into the *same* slot are already
  serialized by the consumption flow control.
"""

from contextlib import ExitStack

import ml_dtypes
import numpy as np

import concourse.bass as bass
from concourse import mybir
from concourse.bass_utils import run_bass_kernel_spmd

BF16 = ml_dtypes.bfloat16
FP8 = ml_dtypes.float8_e4m3

# Problem constants (hardcoded per the harness contract).
N, C, A, R = 4, 128, 180, 184
H = W = 128
NC = N * C  # 512
NCORES = 8
NY = H  # output rows, one PSUM accumulation group each
NBANK = 8  # PSUM banks
NWRING = 12  # weight ring depth (y slots)
NOBUF = 8  # output staging buffers

_cache = {}


def _rho_table():
    """r[k, y, x] int32 rho index; always in [0, R) for this geometry."""
    if "r" not in _cache:
        k = np.arange(A)
        theta = k * (np.pi / A)
        cos_t, sin_t = np.cos(theta), np.sin(theta)
        y, x = np.meshgrid(np.arange(H), np.arange(W), indexing="ij")
        xc = (x - W // 2).astype(np.float64)
        yc = (y - H // 2).astype(np.float64)
        r = np.round(cos_t[:, None, None] * xc[None] + sin_t[:, None, None] * yc[None])
        r = r.astype(np.int64) + R // 2
        assert (r >= 0).all() and (r < R).all()
        _cache["r"] = r.astype(np.int32)
    return _cache["r"]


def _geometry():
    """Static geometry: per-core bin plan + DMA schedule (instruction
    stream identical across cores; only data differs)."""
    if "geo" in _cache:
        return _cache["geo"]
    r = _rho_table()
    lo = r.min(axis=2)  # [A, H]
    hi = r.max(axis=2)

    def win_at_g(k, g):
        w = 0
        for b in range(0, NY, g):
            w = max(w, int(hi[k, b : b + g].max() - lo[k, b : b + g].min()) + 1)
        return w

    gk = {}
    for k in range(A):
        for g in (16, 8, 4, 2):
            if win_at_g(k, g) <= 128:
                gk[k] = g
                break
        assert k in gk

    # FFD bin packing per granularity class.
    def ffd(items):
        bins = []
        for w, k in sorted(items, reverse=True):
            for b in bins:
                if b[0] + w <= 128:
                    b[0] += w
                    b[1].append((k, w))
                    break
            else:
                bins.append([w, [(k, w)]])
        return [b[1] for b in bins]

    # Pack each granularity class, then deal ALL bins sorted finest-g
    # first into groups of 8 (one per core); a position's refresh rate is
    # the finest g in its group (finer refresh of a coarser lane is
    # always valid - the window only shrinks).  Positions are then ordered
    # coarse-g first so fine-g slab waits land late in each y's MM group.
    all_bins = []  # (g, lanes)
    for g in (2, 4, 8, 16):
        items = [(win_at_g(k, g), k) for k in range(A) if gk[k] == g]
        all_bins += [(g, b) for b in ffd(items)]
    while len(all_bins) % NCORES:
        all_bins.append((16, []))
    P = len(all_bins) // NCORES
    groups = sorted(
        (all_bins[j * NCORES : (j + 1) * NCORES] for j in range(P)),
        key=lambda grp: -min(g for g, _ in grp),
    )
    profile = [min(g for g, _ in grp) for grp in groups]
    # Slab buffers per position: deep rings for fine-g positions so their
    # frequent refresh gates release far ahead of consumption.
    nbuf = [4 if g >= 16 else 8 for g in profile]

    # lanes[c][i] = list of (k, width, base_row); bases are prefix sums.
    lanes = [[] for _ in range(NCORES)]
    for c in range(NCORES):
        for grp in groups:
            _, lane_list = grp[c]
            out, base = [], 0
            for k, w in lane_list:
                out.append((k, w, base))
                base += w
            assert base <= 128
            lanes[c].append(out)

    # Slab slots: position i has NY // g_i blocks.
    slot_of = {}
    nslot = 0
    for i, g in enumerate(profile):
        for b in range(NY // g):
            slot_of[(i, b)] = nslot
            nslot += 1

    # DMA schedule sorted by issue key: (key, tie, kind, i, b, flow_wait).
    # Slab blocks are placed early in the stream so block-boundary bursts
    # are not stuck behind weight chunks whose flow waits release later.
    items = []
    for i, g in enumerate(profile):
        for b in range(NY // g):
            fw = (b - nbuf[i] + 1) * g if b >= nbuf[i] else None
            # key <= deadline (b*g) and key > fw: every item the flow wait
            # depends on sorts earlier -> deadlock-free.
            key = max(0 if fw is None else fw + 1, b * g - NWRING // 2)
            items.append((key, 1, "slab", i, b, fw))
    for y in range(NY):
        fw = y - NWRING + 1 if y >= NWRING else None
        items.append((y, 0, "wt", y, 0, fw))
    items.sort(key=lambda t: (t[0], t[1]))

    def lane_off(k, width, g, b):
        l = int(lo[k, b * g : (b + 1) * g].min())
        h = int(hi[k, b * g : (b + 1) * g].max())
        assert h - l + 1 <= width
        return min(l, R - width)

    _cache["geo"] = dict(
        profile=profile, P=P, lanes=lanes, slot_of=slot_of, nslot=nslot,
        items=items, lane_off=lane_off, nbuf=nbuf,
        slab_base=np.concatenate([[0], np.cumsum(nbuf)]).tolist(),
    )
    return _cache["geo"]


def _host_tables():
    """Per-core one-hot weight tables (geometry only; cached across calls)
    and slab assembly metadata."""
    if "wts" in _cache:
        return _cache["wts"], _cache["slab_meta"]
    geo = _geometry()
    r = _rho_table()
    P = geo["P"]
    profile = geo["profile"]
    wts = []
    slab_meta = []  # per core: list over slots of [(k, o, width, base)]
    xs = np.arange(W)
    ys = np.arange(NY)
    for c in range(NCORES):
        w = np.zeros((NY, 128, P * 128), BF16)
        meta = [[] for _ in range(geo["nslot"])]
        for i, g in enumerate(profile):
            for k, width, base in geo["lanes"][c][i]:
                for b in range(NY // g):
                    o = geo["lane_off"](k, width, g, b)
                    meta[geo["slot_of"][(i, b)]].append((k, o, width, base))
                    yb = ys[b * g : (b + 1) * g]
                    rowidx = r[k, yb] - o + base  # [g, W]
                    w[yb[:, None], rowidx, i * 128 + xs[None, :]] = 1
        wts.append(w)
        slab_meta.append(meta)
    _cache["wts"] = wts
    _cache["slab_meta"] = slab_meta
    return wts, slab_meta


def _build_nc():
    if "nc" in _cache:
        return _cache["nc"]
    geo = _geometry()
    P = geo["P"]
    profile = geo["profile"]
    nslot = geo["nslot"]

    nc = bass.Bass("TRN2", debug=False, target_bir_lowering=False, num_devices=NCORES)
    wts_d = nc.dram_tensor(
        "wts", [NY, 128, P * 128], mybir.dt.bfloat16, kind="ExternalInput"
    ).ap()
    slab_d = nc.dram_tensor(
        "slabs", [nslot, 128, NC], mybir.dt.bfloat16, kind="ExternalInput"
    ).ap()
    out_d = nc.dram_tensor(
        "out", [NY, 128, NC], mybir.dt.float32, kind="ExternalOutput"
    ).ap()

    ctx = ExitStack()
    _cache["ctx"] = ctx
    SLABCOLS = geo["slab_base"][P] * NC
    slabs_sb = ctx.enter_context(
        nc.sbuf_tensor("slabs_sb", [128, SLABCOLS], mybir.dt.bfloat16)
    )
    wring = ctx.enter_context(
        nc.sbuf_tensor("wring", [128, NWRING * P * 128], mybir.dt.bfloat16)
    )
    obuf = ctx.enter_context(
        nc.sbuf_tensor("obuf", [128, NOBUF * NC], mybir.dt.float32)
    )
    ps = [
        ctx.enter_context(nc.psum_tensor(f"ps{i}", [128, NC], mybir.dt.float32))
        for i in range(NBANK)
    ]
    mm_sem = ctx.enter_context(nc.semaphore("mm_sem"))
    cp_sem = ctx.enter_context(nc.semaphore("cp_sem"))
    dump_sem = ctx.enter_context(nc.semaphore("dump_sem"))
    wt_sems = [
        ctx.enter_context(nc.semaphore(f"wt{s}")) for s in range(NWRING)
    ]
    sl_sems = [
        [ctx.enter_context(nc.semaphore(f"sl{i}_{h}")) for h in range(geo["nbuf"][i])]
        for i in range(P)
    ]
    block = ctx.enter_context(nc.Block())

    def slab_col(i, buf):
        return (geo["slab_base"][i] + buf) * NC

    @block.gpsimd
    def _(gpsimd):
        for _, _, kind, i, b, fw in geo["items"]:
            if fw is not None:
                gpsimd.wait_ge(mm_sem, fw)
            if kind == "slab":
                nb = geo["nbuf"][i]
                col = slab_col(i, b % nb)
                gpsimd.dma_start(
                    slabs_sb[:, col : col + NC], slab_d[geo["slot_of"][(i, b)]]
                ).then_inc(sl_sems[i][b % nb], 16)
            else:
                y = i
                base = (y % NWRING) * P * 128
                gpsimd.dma_start(
                    wring[:, base : base + P * 128], wts_d[y]
                ).then_inc(wt_sems[y % NWRING], 16)

    @block.tensor
    def _(tensor):
        # Warm the PE HAM clock gate during the DMA prologue with junk
        # matmuls (quiet SBUF regions; bank 7 is cleared by y=7's start).
        wq = (NWRING - 1) * P * 128
        sq = SLABCOLS - NC
        for _ in range(40):
            tensor.matmul(
                out=ps[NBANK - 1][:, :128],
                lhsT=wring[:, wq : wq + 128],
                rhs=slabs_sb[:, sq : sq + 128],
                start=True,
                stop=True,
            )
        for y in range(NY):
            if y >= NBANK:
                tensor.wait_ge(cp_sem, y - NBANK + 1)
            tensor.wait_ge(wt_sems[y % NWRING], 16 * (y // NWRING + 1))
            wbase = (y % NWRING) * P * 128
            for i, g in enumerate(profile):
                nb = geo["nbuf"][i]
                b = y // g
                if y % g == 0:
                    tensor.wait_ge(sl_sems[i][b % nb], 16 * (b // nb + 1))
                col = slab_col(i, b % nb)
                mm = tensor.matmul(
                    out=ps[y % NBANK][:],
                    lhsT=wring[:, wbase + i * 128 : wbase + (i + 1) * 128],
                    rhs=slabs_sb[:, col : col + NC],
                    start=(i == 0),
                    stop=(i == P - 1),
                )
            mm.then_inc(mm_sem, 1)

    @block.scalar
    def _(scalar):
        for y in range(NY):
            scalar.wait_ge(mm_sem, y + 1)
            if y >= NOBUF:
                scalar.wait_ge(dump_sem, 16 * (y - NOBUF + 1))
            col = (y % NOBUF) * NC
            scalar.copy(obuf[:, col : col + NC], ps[y % NBANK][:]).then_inc(cp_sem, 1)

    @block.sync
    def _(sync):
        for y in range(NY):
            sync.wait_ge(cp_sem, y + 1)
            col = (y % NOBUF) * NC
            sync.dma_start(out_d[y], obuf[:, col : col + NC]).then_inc(dump_sem, 16)

    _cache["nc"] = nc
    return nc


def _install_ntff_hook():
    """Provide the antenv.axon_hooks shim the image lacks, wiring the
    ctypes NTFF profiler from trn_agent_boot."""
    import sys
    import types

    if "antenv.axon_hooks" in sys.modules:
        return
    import antenv
    from trn_agent_boot.trn_boot import _ntff_profile_via_ctypes

    mod = types.ModuleType("antenv.axon_hooks")
    hook = _ntff_profile_via_ctypes("/opt/axon/libaxon_pjrt.so")
    mod.get_axon_ntff_profile_hook = lambda: hook
    mod.set_axon_ntff_profile_hook = lambda h: None
    sys.modules["antenv.axon_hooks"] = mod
    antenv.axon_hooks = mod


def hw_exec_time_ns(trace_cores=None):
    """Re-run the last kernel() invocation with tracing; return max core ns."""
    _install_ntff_hook()
    nc = _cache["nc"]
    res = run_bass_kernel_spmd(
        nc,
        _cache["in_maps"],
        core_ids=list(range(NCORES)),
        trace=True,
        trace_cores=trace_cores,
    )
    _cache["trace"] = res
    return res.exec_time_ns


def kernel(accumulator, out_H=128, out_W=128, numangle=180, numrho=184):
    accumulator = np.asarray(accumulator, np.float32)
    assert accumulator.shape == (N, C, A, R), accumulator.shape
    assert int(out_H) == H and int(out_W) == W
    assert int(numangle) == A and int(numrho) == R

    geo = _geometry()
    wts, slab_meta = _host_tables()
    nc = _build_nc()

    # acc_t[k, rho, nc] bf16 - slab source.
    acc_t = np.ascontiguousarray(
        accumulator.reshape(NC, A, R).transpose(1, 2, 0)
    ).astype(BF16)

    in_maps = []
    for c in range(NCORES):
        slabs = np.zeros((geo["nslot"], 128, NC), BF16)
        for slot, entries in enumerate(slab_meta[c]):
            for k, o, width, base in entries:
                slabs[slot, base : base + width] = acc_t[k, o : o + width]
        in_maps.append({"wts": wts[c], "slabs": slabs})
    _cache["in_maps"] = in_maps
    res = run_bass_kernel_spmd(nc, in_maps, core_ids=list(range(NCORES)))

    # Unshard: sum the 8 per-core partials.  out[y, x, nc]
    total = np.zeros((NY, 128, NC), np.float64)
    for c in range(NCORES):
        total += res.results[c]["out"]
    return (
        total.transpose(2, 0, 1).reshape(N, C, H, W).astype(np.float32)
    )


# revision 20
# speedup vs baseline: 1.7977x; 1.0082x over previous
"""Trainium2 Bass kernel for the inverse deep-hough-transform gather-reduce.

out[n, c, y, x] = sum_k acc[n, c, k, rho_idx[k, y, x]]

Design (v4): one-hot matmul gather on the PE (tensor engine)
------------------------------------------------------------
For a fixed output row y and angle k, the gather over x is a selection
matmul:  out[x, nc] += sum_rho OH[rho, x] * acc_k[rho, nc], with OH the
0/1 one-hot of rho == r(k, y, x).  The PE streams the 512 nc columns at
1 col/cycle and produces >= 128 gathered elements per cycle.

- Contraction dim K packs multiple angles' rho *windows* (bin packing):
  angle k needs a window of win_g(k) rho rows covering a y-block of g(k)
  rows (g in {16,8,4,2} per angle; finer g for |cos| ~ 1 angles whose
  window drifts fast with y).  First-fit-decreasing packs the windows
  into 128-row bins; one bin = one matmul per y, summing all its angles.
- Sharding: bins are dealt across the 8 cores class-by-class so the SPMD
  instruction stream is identical on every core; all per-core geometry
  lives in host-built data (one-hot weight tiles + rho window "slabs").
  Host sums the 8 per-core partial outputs.
- Per y: P (~18) accumulating matmuls into one PSUM bank (8 banks
  cycle), ACT evicts PSUM->SBUF, sync DMAs the row out to HBM.  Weight
  tiles and slab blocks stream HBM->SBUF on the gpsimd queue.
- Sync uses one semaphore per SBUF slot (weight-ring slot / slab
  double-buffer half) so correctness does not depend on cross-DMA
  completion ordering: successive DMAs into the *same* slot are already
  serialized by the consumption flow control.
"""

from contextlib import ExitStack

import ml_dtypes
import numpy as np

import concourse.bass as bass
from concourse import mybir
from concourse.bass_utils import run_bass_kernel_spmd

BF16 = ml_dtypes.bfloat16
FP8 = ml_dtypes.float8_e4m3

# Problem constants (hardcoded per the harness contract).
N, C, A, R = 4, 128, 180, 184
H = W = 128
NC = N * C  # 512
NCORES = 8
NY = H  # output rows, one PSUM accumulation group each
NBANK = 8  # PSUM banks
NWRING = 12  # weight ring depth (y slots)
NOBUF = 8  # output staging buffers

_cache = {}


def _rho_table():
    """r[k, y, x] int32 rho index; always in [0, R) for this geometry."""
    if "r" not in _cache:
        k = np.arange(A)
        theta = k * (np.pi / A)
        cos_t, sin_t = np.cos(theta), np.sin(theta)
        y, x = np.meshgrid(np.arange(H), np.arange(W), indexing="ij")
        xc = (x - W // 2).astype(np.float64)
        yc = (y - H // 2).astype(np.float64)
        r = np.round(cos_t[:, None, None] * xc[None] + sin_t[:, None, None] * yc[None])
        r = r.astype(np.int64) + R // 2
        assert (r >= 0).all() and (r < R).all()
        _cache["r"] = r.astype(np.int32)
    return _cache["r"]


def _geometry():
    """Static geometry: per-core bin plan + DMA schedule (instruction
    stream identical across cores; only data differs)."""
    if "geo" in _cache:
        return _cache["geo"]
    r = _rho_table()
    lo = r.min(axis=2)  # [A, H]
    hi = r.max(axis=2)

    def win_at_g(k, g):
        w = 0
        for b in range(0, NY, g):
            w = max(w, int(hi[k, b : b + g].max() - lo[k, b : b + g].min()) + 1)
        return w

    gk = {}
    for k in range(A):
        for g in (16, 8, 4, 2):
            if win_at_g(k, g) <= 128:
                gk[k] = g
                break
        assert k in gk

    # FFD bin packing per granularity class.
    def ffd(items):
        bins = []
        for w, k in sorted(items, reverse=True):
            for b in bins:
                if b[0] + w <= 128:
                    b[0] += w
                    b[1].append((k, w))
                    break
            else:
                bins.append([w, [(k, w)]])
        return [b[1] for b in bins]

    # Pack each granularity class, then deal ALL bins sorted finest-g
    # first into groups of 8 (one per core); a position's refresh rate is
    # the finest g in its group (finer refresh of a coarser lane is
    # always valid - the window only shrinks).  Positions are then ordered
    # coarse-g first so fine-g slab waits land late in each y's MM group.
    all_bins = []  # (g, lanes)
    for g in (2, 4, 8, 16):
        items = [(win_at_g(k, g), k) for k in range(A) if gk[k] == g]
        all_bins += [(g, b) for b in ffd(items)]
    while len(all_bins) % NCORES:
        all_bins.append((16, []))
    P = len(all_bins) // NCORES
    groups = sorted(
        (all_bins[j * NCORES : (j + 1) * NCORES] for j in range(P)),
        key=lambda grp: -min(g for g, _ in grp),
    )
    profile = [min(g for g, _ in grp) for grp in groups]
    # Slab buffers per position: deep rings for fine-g positions so their
    # frequent refresh gates release far ahead of consumption.
    nbuf = [4 if g >= 16 else 8 for g in profile]

    # lanes[c][i] = list of (k, width, base_row); bases are prefix sums.
    lanes = [[] for _ in range(NCORES)]
    for c in range(NCORES):
        for grp in groups:
            _, lane_list = grp[c]
            out, base = [], 0
            for k, w in lane_list:
                out.append((k, w, base))
                base += w
            assert base <= 128
            lanes[c].append(out)

    # Slab slots: position i has NY // g_i blocks.
    slot_of = {}
    nslot = 0
    for i, g in enumerate(profile):
        for b in range(NY // g):
            slot_of[(i, b)] = nslot
            nslot += 1

    # DMA schedule sorted by issue key: (key, tie, kind, i, b, flow_wait).
    # Slab blocks are placed early in the stream so block-boundary bursts
    # are not stuck behind weight chunks whose flow waits release later.
    # Positions whose block-0 slab ships as one batched prologue DMA
    # (uniform SBUF stride): the contiguous run of nbuf=4 positions.
    batch0 = [i for i, g in enumerate(profile) if nbuf[i] == 4]
    assert batch0 == list(range(len(batch0)))

    items = [(0, 0.5, "slab0", 0, 0, None)]
    for i, g in enumerate(profile):
        for b in range(NY // g):
            if b == 0 and i in batch0:
                continue
            fw = (b - nbuf[i] + 1) * g if b >= nbuf[i] else None
            # key <= deadline (b*g) and key > fw: every item the flow wait
            # depends on sorts earlier -> deadlock-free.
            key = max(0 if fw is None else fw + 1, b * g - NWRING // 2)
            items.append((key, 1, "slab", i, b, fw))
    for y in range(NY):
        fw = y - NWRING + 1 if y >= NWRING else None
        items.append((y, 0, "wt", y, 0, fw))
    items.sort(key=lambda t: (t[0], t[1]))

    def lane_off(k, width, g, b):
        l = int(lo[k, b * g : (b + 1) * g].min())
        h = int(hi[k, b * g : (b + 1) * g].max())
        assert h - l + 1 <= width
        return min(l, R - width)

    _cache["geo"] = dict(
        profile=profile, P=P, lanes=lanes, slot_of=slot_of, nslot=nslot,
        items=items, lane_off=lane_off, nbuf=nbuf, batch0=batch0,
        slab_base=np.concatenate([[0], np.cumsum(nbuf)]).tolist(),
    )
    return _cache["geo"]


def _host_tables():
    """Per-core one-hot weight tables (geometry only; cached across calls)
    and slab assembly metadata."""
    if "wts" in _cache:
        return _cache["wts"], _cache["slab_meta"]
    geo = _geometry()
    r = _rho_table()
    P = geo["P"]
    profile = geo["profile"]
    wts = []
    slab_meta = []  # per core: list over slots of [(k, o, width, base)]
    xs = np.arange(W)
    ys = np.arange(NY)
    for c in range(NCORES):
        w = np.zeros((NY, 128, P * 128), BF16)
        meta = [[] for _ in range(geo["nslot"])]
        for i, g in enumerate(profile):
            for k, width, base in geo["lanes"][c][i]:
                for b in range(NY // g):
                    o = geo["lane_off"](k, width, g, b)
                    meta[geo["slot_of"][(i, b)]].append((k, o, width, base))
                    yb = ys[b * g : (b + 1) * g]
                    rowidx = r[k, yb] - o + base  # [g, W]
                    w[yb[:, None], rowidx, i * 128 + xs[None, :]] = 1
        wts.append(w)
        slab_meta.append(meta)
    _cache["wts"] = wts
    _cache["slab_meta"] = slab_meta
    return wts, slab_meta


def _build_nc():
    if "nc" in _cache:
        return _cache["nc"]
    geo = _geometry()
    P = geo["P"]
    profile = geo["profile"]
    nslot = geo["nslot"]

    nc = bass.Bass("TRN2", debug=False, target_bir_lowering=False, num_devices=NCORES)
    wts_d = nc.dram_tensor(
        "wts", [NY, 128, P * 128], mybir.dt.bfloat16, kind="ExternalInput"
    ).ap()
    slab_d = nc.dram_tensor(
        "slabs", [nslot, 128, NC], mybir.dt.bfloat16, kind="ExternalInput"
    ).ap()
    nb0 = len(geo["batch0"])
    slab0_d = nc.dram_tensor(
        "slab0", [128, nb0, NC], mybir.dt.bfloat16, kind="ExternalInput"
    ).ap()
    out_d = nc.dram_tensor(
        "out", [NY, 128, NC], mybir.dt.float32, kind="ExternalOutput"
    ).ap()

    ctx = ExitStack()
    _cache["ctx"] = ctx
    SLABCOLS = geo["slab_base"][P] * NC
    slabs_sb = ctx.enter_context(
        nc.sbuf_tensor("slabs_sb", [128, SLABCOLS], mybir.dt.bfloat16)
    )
    wring = ctx.enter_context(
        nc.sbuf_tensor("wring", [128, NWRING * P * 128], mybir.dt.bfloat16)
    )
    obuf = ctx.enter_context(
        nc.sbuf_tensor("obuf", [128, NOBUF * NC], mybir.dt.float32)
    )
    ps = [
        ctx.enter_context(nc.psum_tensor(f"ps{i}", [128, NC], mybir.dt.float32))
        for i in range(NBANK)
    ]
    mm_sem = ctx.enter_context(nc.semaphore("mm_sem"))
    cp_sem = ctx.enter_context(nc.semaphore("cp_sem"))
    dump_sem = ctx.enter_context(nc.semaphore("dump_sem"))
    wt_sems = [
        ctx.enter_context(nc.semaphore(f"wt{s}")) for s in range(NWRING)
    ]
    b0_sem = ctx.enter_context(nc.semaphore("b0_sem"))
    sl_sems = [
        [ctx.enter_context(nc.semaphore(f"sl{i}_{h}")) for h in range(geo["nbuf"][i])]
        for i in range(P)
    ]
    block = ctx.enter_context(nc.Block(no_gpsimd_drain=True))

    def slab_col(i, buf):
        return (geo["slab_base"][i] + buf) * NC

    @block.gpsimd
    def _(gpsimd):
        for _, _, kind, i, b, fw in geo["items"]:
            if fw is not None:
                gpsimd.wait_ge(mm_sem, fw)
            if kind == "slab0":
                # batched block-0 slabs for the nbuf=4 positions: SBUF
                # buffer-0 columns are a uniform 4*NC stride apart.
                dst = slabs_sb[:, : nb0 * 4 * NC].rearrange(
                    "p (i n) -> p i n", n=4 * NC
                )[:, :, :NC]
                gpsimd.dma_start(dst, slab0_d[:]).then_inc(b0_sem, 16)
            elif kind == "slab":
                nb = geo["nbuf"][i]
                col = slab_col(i, b % nb)
                gpsimd.dma_start(
                    slabs_sb[:, col : col + NC], slab_d[geo["slot_of"][(i, b)]]
                ).then_inc(sl_sems[i][b % nb], 16)
            else:
                y = i
                base = (y % NWRING) * P * 128
                gpsimd.dma_start(
                    wring[:, base : base + P * 128], wts_d[y]
                ).then_inc(wt_sems[y % NWRING], 16)

    @block.tensor
    def _(tensor):
        # Warm the PE HAM clock gate during the DMA prologue with junk
        # matmuls (quiet SBUF regions; bank 7 is cleared by y=7's start).
        wq = (NWRING - 1) * P * 128
        sq = SLABCOLS - NC
        for _ in range(40):
            tensor.matmul(
                out=ps[NBANK - 1][:, :128],
                lhsT=wring[:, wq : wq + 128],
                rhs=slabs_sb[:, sq : sq + 128],
                start=True,
                stop=True,
            )
        for y in range(NY):
            if y >= NBANK:
                tensor.wait_ge(cp_sem, y - NBANK + 1)
            tensor.wait_ge(wt_sems[y % NWRING], 16 * (y // NWRING + 1))
            wbase = (y % NWRING) * P * 128
            for i, g in enumerate(profile):
                nb = geo["nbuf"][i]
                b = y // g
                if y % g == 0:
                    if i in geo["batch0"] and b == 0:
                        tensor.wait_ge(b0_sem, 16)
                    elif i in geo["batch0"] and b % nb == 0:
                        tensor.wait_ge(sl_sems[i][0], 16 * (b // nb))
                    else:
                        tensor.wait_ge(sl_sems[i][b % nb], 16 * (b // nb + 1))
                col = slab_col(i, b % nb)
                mm = tensor.matmul(
                    out=ps[y % NBANK][:],
                    lhsT=wring[:, wbase + i * 128 : wbase + (i + 1) * 128],
                    rhs=slabs_sb[:, col : col + NC],
                    start=(i == 0),
                    stop=(i == P - 1),
                )
            mm.then_inc(mm_sem, 1)

    @block.scalar
    def _(scalar):
        for y in range(NY):
            scalar.wait_ge(mm_sem, y + 1)
            if y >= NOBUF:
                scalar.wait_ge(dump_sem, 16 * (y - NOBUF + 1))
            col = (y % NOBUF) * NC
            scalar.copy(obuf[:, col : col + NC], ps[y % NBANK][:]).then_inc(cp_sem, 1)

    @block.sync
    def _(sync):
        for y in range(NY):
            sync.wait_ge(cp_sem, y + 1)
            col = (y % NOBUF) * NC
            sync.dma_start(out_d[y], obuf[:, col : col + NC]).then_inc(dump_sem, 16)

    _cache["nc"] = nc
    return nc


def _install_ntff_hook():
    """Provide the antenv.axon_hooks shim the image lacks, wiring the
    ctypes NTFF profiler from trn_agent_boot."""
    import sys
    import types

    if "antenv.axon_hooks" in sys.modules:
        return
    import antenv
    from trn_agent_boot.trn_boot import _ntff_profile_via_ctypes

    mod = types.ModuleType("antenv.axon_hooks")
    hook = _ntff_profile_via_ctypes("/opt/axon/libaxon_pjrt.so")
    mod.get_axon_ntff_profile_hook = lambda: hook
    mod.set_axon_ntff_profile_hook = lambda h: None
    sys.modules["antenv.axon_hooks"] = mod
    antenv.axon_hooks = mod


def hw_exec_time_ns(trace_cores=None):
    """Re-run the last kernel() invocation with tracing; return max core ns."""
    _install_ntff_hook()
    nc = _cache["nc"]
    res = run_bass_kernel_spmd(
        nc,
        _cache["in_maps"],
        core_ids=list(range(NCORES)),
        trace=True,
        trace_cores=trace_cores,
    )
    _cache["trace"] = res
    return res.exec_time_ns


def kernel(accumulator, out_H=128, out_W=128, numangle=180, numrho=184):
    accumulator = np.asarray(accumulator, np.float32)
    assert accumulator.shape == (N, C, A, R), accumulator.shape
    assert int(out_H) == H and int(out_W) == W
    assert int(numangle) == A and int(numrho) == R

    geo = _geometry()
    wts, slab_meta = _host_tables()
    nc = _build_nc()

    # acc_t[k, rho, nc] bf16 - slab source.
    acc_t = np.ascontiguousarray(
        accumulator.reshape(NC, A, R).transpose(1, 2, 0)
    ).astype(BF16)

    in_maps = []
    for c in range(NCORES):
        slabs = np.zeros((geo["nslot"], 128, NC), BF16)
        for slot, entries in enumerate(slab_meta[c]):
            for k, o, width, base in entries:
                slabs[slot, base : base + width] = acc_t[k, o : o + width]
        slab0 = np.ascontiguousarray(
            slabs[[geo["slot_of"][(i, 0)] for i in geo["batch0"]]]
            .transpose(1, 0, 2)
        )
        in_maps.append({"wts": wts[c], "slabs": slabs, "slab0": slab0})
    _cache["in_maps"] = in_maps
    res = run_bass_kernel_spmd(nc, in_maps, core_ids=list(range(NCORES)))

    # Unshard: sum the 8 per-core partials.  out[y, x, nc]
    total = np.zeros((NY, 128, NC), np.float64)
    for c in range(NCORES):
        total += res.results[c]["out"]
    return (
        total.transpose(2, 0, 1).reshape(N, C, H, W).astype(np.float32)
    )


# revision 22
# speedup vs baseline: 1.8035x; 1.0032x over previous
"""Trainium2 Bass kernel for the inverse deep-hough-transform gather-reduce.

out[n, c, y, x] = sum_k acc[n, c, k, rho_idx[k, y, x]]

Design (v4): one-hot matmul gather on the PE (tensor engine)
------------------------------------------------------------
For a fixed output row y and angle k, the gather over x is a selection
matmul:  out[x, nc] += sum_rho OH[rho, x] * acc_k[rho, nc], with OH the
0/1 one-hot of rho == r(k, y, x).  The PE streams the 512 nc columns at
1 col/cycle and produces >= 128 gathered elements per cycle.

- Contraction dim K packs multiple angles' rho *windows* (bin packing):
  angle k needs a window of win_g(k) rho rows covering a y-block of g(k)
  rows (g in {16,8,4,2} per angle; finer g for |cos| ~ 1 angles whose
  window drifts fast with y).  First-fit-decreasing packs the windows
  into 128-row bins; one bin = one matmul per y, summing all its angles.
- Sharding: bins are dealt across the 8 cores class-by-class so the SPMD
  instruction stream is identical on every core; all per-core geometry
  lives in host-built data (one-hot weight tiles + rho window "slabs").
  Host sums the 8 per-core partial outputs.
- Per y: P (~18) accumulating matmuls into one PSUM bank (8 banks
  cycle), ACT evicts PSUM->SBUF, sync DMAs the row out to HBM.  Weight
  tiles and slab blocks stream HBM->SBUF on the gpsimd queue.
- Sync uses one semaphore per SBUF slot (weight-ring slot / slab
  double-buffer half) so correctness does not depend on cross-DMA
  completion ordering: successive DMAs into the *same* slot are already
  serialized by the consumption flow control.
"""

from contextlib import ExitStack

import ml_dtypes
import numpy as np

import concourse.bass as bass
from concourse import mybir
from concourse.bass_utils import run_bass_kernel_spmd

BF16 = ml_dtypes.bfloat16
FP8 = ml_dtypes.float8_e4m3

# Problem constants (hardcoded per the harness contract).
N, C, A, R = 4, 128, 180, 184
H = W = 128
NC = N * C  # 512
NCORES = 8
NY = H  # output rows, one PSUM accumulation group each
NBANK = 8  # PSUM banks
NWRING = 12  # weight ring depth (y slots)
NOBUF = 8  # output staging buffers

_cache = {}


def _rho_table():
    """r[k, y, x] int32 rho index; always in [0, R) for this geometry."""
    if "r" not in _cache:
        k = np.arange(A)
        theta = k * (np.pi / A)
        cos_t, sin_t = np.cos(theta), np.sin(theta)
        y, x = np.meshgrid(np.arange(H), np.arange(W), indexing="ij")
        xc = (x - W // 2).astype(np.float64)
        yc = (y - H // 2).astype(np.float64)
        r = np.round(cos_t[:, None, None] * xc[None] + sin_t[:, None, None] * yc[None])
        r = r.astype(np.int64) + R // 2
        assert (r >= 0).all() and (r < R).all()
        _cache["r"] = r.astype(np.int32)
    return _cache["r"]


def _geometry():
    """Static geometry: per-core bin plan + DMA schedule (instruction
    stream identical across cores; only data differs)."""
    if "geo" in _cache:
        return _cache["geo"]
    r = _rho_table()
    lo = r.min(axis=2)  # [A, H]
    hi = r.max(axis=2)

    def win_at_g(k, g):
        w = 0
        for b in range(0, NY, g):
            w = max(w, int(hi[k, b : b + g].max() - lo[k, b : b + g].min()) + 1)
        return w

    gk = {}
    for k in range(A):
        for g in (16, 8, 4, 2):
            if win_at_g(k, g) <= 128:
                gk[k] = g
                break
        assert k in gk

    # FFD bin packing per granularity class.
    def ffd(items):
        bins = []
        for w, k in sorted(items, reverse=True):
            for b in bins:
                if b[0] + w <= 128:
                    b[0] += w
                    b[1].append((k, w))
                    break
            else:
                bins.append([w, [(k, w)]])
        return [b[1] for b in bins]

    # Pack each granularity class, then deal ALL bins sorted finest-g
    # first into groups of 8 (one per core); a position's refresh rate is
    # the finest g in its group (finer refresh of a coarser lane is
    # always valid - the window only shrinks).  Positions are then ordered
    # coarse-g first so fine-g slab waits land late in each y's MM group.
    all_bins = []  # (g, lanes)
    for g in (2, 4, 8, 16):
        items = [(win_at_g(k, g), k) for k in range(A) if gk[k] == g]
        all_bins += [(g, b) for b in ffd(items)]
    while len(all_bins) % NCORES:
        all_bins.append((16, []))
    P = len(all_bins) // NCORES
    groups = sorted(
        (all_bins[j * NCORES : (j + 1) * NCORES] for j in range(P)),
        key=lambda grp: -min(g for g, _ in grp),
    )
    profile = [min(g for g, _ in grp) for grp in groups]
    # Slab buffers per position: deep rings for fine-g positions so their
    # frequent refresh gates release far ahead of consumption.
    nbuf = [4 if g >= 16 else 8 for g in profile]

    # lanes[c][i] = list of (k, width, base_row); bases are prefix sums.
    lanes = [[] for _ in range(NCORES)]
    for c in range(NCORES):
        for grp in groups:
            _, lane_list = grp[c]
            out, base = [], 0
            for k, w in lane_list:
                out.append((k, w, base))
                base += w
            assert base <= 128
            lanes[c].append(out)

    # Slab slots: position i has NY // g_i blocks.
    slot_of = {}
    nslot = 0
    for i, g in enumerate(profile):
        for b in range(NY // g):
            slot_of[(i, b)] = nslot
            nslot += 1

    # DMA schedule sorted by issue key: (key, tie, kind, i, b, flow_wait).
    # Slab blocks are placed early in the stream so block-boundary bursts
    # are not stuck behind weight chunks whose flow waits release later.
    # Positions whose block-0 slab ships as one batched prologue DMA
    # (uniform SBUF stride): the contiguous run of nbuf=4 positions.
    batch0 = [i for i, g in enumerate(profile) if nbuf[i] == 4]
    assert batch0 == list(range(len(batch0)))

    items = [(0, 0.5, "slab0", 0, 0, None)]
    for i, g in enumerate(profile):
        for b in range(NY // g):
            if b == 0 and i in batch0:
                continue
            fw = (b - nbuf[i] + 1) * g if b >= nbuf[i] else None
            # key <= deadline (b*g) and key > fw: every item the flow wait
            # depends on sorts earlier -> deadlock-free.
            key = max(0 if fw is None else fw + 1, b * g - NWRING // 2)
            items.append((key, 1, "slab", i, b, fw))
    for y in range(NY):
        fw = y - NWRING + 1 if y >= NWRING else None
        items.append((y, 0, "wt", y, 0, fw))
    items.sort(key=lambda t: (t[0], t[1]))

    def lane_off(k, width, g, b):
        l = int(lo[k, b * g : (b + 1) * g].min())
        h = int(hi[k, b * g : (b + 1) * g].max())
        assert h - l + 1 <= width
        return min(l, R - width)

    _cache["geo"] = dict(
        profile=profile, P=P, lanes=lanes, slot_of=slot_of, nslot=nslot,
        items=items, lane_off=lane_off, nbuf=nbuf, batch0=batch0,
        slab_base=np.concatenate([[0], np.cumsum(nbuf)]).tolist(),
    )
    return _cache["geo"]


def _host_tables():
    """Per-core one-hot weight tables (geometry only; cached across calls)
    and slab assembly metadata."""
    if "wts" in _cache:
        return _cache["wts"], _cache["slab_meta"]
    geo = _geometry()
    r = _rho_table()
    P = geo["P"]
    profile = geo["profile"]
    wts = []
    slab_meta = []  # per core: list over slots of [(k, o, width, base)]
    xs = np.arange(W)
    ys = np.arange(NY)
    for c in range(NCORES):
        w = np.zeros((NY, 128, P * 128), BF16)
        meta = [[] for _ in range(geo["nslot"])]
        for i, g in enumerate(profile):
            for k, width, base in geo["lanes"][c][i]:
                for b in range(NY // g):
                    o = geo["lane_off"](k, width, g, b)
                    meta[geo["slot_of"][(i, b)]].append((k, o, width, base))
                    yb = ys[b * g : (b + 1) * g]
                    rowidx = r[k, yb] - o + base  # [g, W]
                    w[yb[:, None], rowidx, i * 128 + xs[None, :]] = 1
        wts.append(w)
        slab_meta.append(meta)
    _cache["wts"] = wts
    _cache["slab_meta"] = slab_meta
    return wts, slab_meta


def _build_nc():
    if "nc" in _cache:
        return _cache["nc"]
    geo = _geometry()
    P = geo["P"]
    profile = geo["profile"]
    nslot = geo["nslot"]

    nc = bass.Bass("TRN2", debug=False, target_bir_lowering=False, num_devices=NCORES)
    wts_d = nc.dram_tensor(
        "wts", [NY, 128, P * 128], mybir.dt.bfloat16, kind="ExternalInput"
    ).ap()
    slab_d = nc.dram_tensor(
        "slabs", [nslot, 128, NC], mybir.dt.bfloat16, kind="ExternalInput"
    ).ap()
    nb0 = len(geo["batch0"])
    slab0_d = nc.dram_tensor(
        "slab0", [128, nb0, NC], mybir.dt.bfloat16, kind="ExternalInput"
    ).ap()
    out_d = nc.dram_tensor(
        "out", [NY, 128, NC], mybir.dt.float32, kind="ExternalOutput"
    ).ap()

    ctx = ExitStack()
    _cache["ctx"] = ctx
    SLABCOLS = geo["slab_base"][P] * NC
    slabs_sb = ctx.enter_context(
        nc.sbuf_tensor("slabs_sb", [128, SLABCOLS], mybir.dt.bfloat16)
    )
    wring = ctx.enter_context(
        nc.sbuf_tensor("wring", [128, NWRING * P * 128], mybir.dt.bfloat16)
    )
    obuf = ctx.enter_context(
        nc.sbuf_tensor("obuf", [128, NOBUF * NC], mybir.dt.float32)
    )
    ps = [
        ctx.enter_context(nc.psum_tensor(f"ps{i}", [128, NC], mybir.dt.float32))
        for i in range(NBANK)
    ]
    mm_sem = ctx.enter_context(nc.semaphore("mm_sem"))
    cp_sem = ctx.enter_context(nc.semaphore("cp_sem"))
    dump_sem = ctx.enter_context(nc.semaphore("dump_sem"))
    wt_sems = [
        ctx.enter_context(nc.semaphore(f"wt{s}")) for s in range(NWRING)
    ]
    b0_sem = ctx.enter_context(nc.semaphore("b0_sem"))
    sl_sems = [
        [ctx.enter_context(nc.semaphore(f"sl{i}_{h}")) for h in range(geo["nbuf"][i])]
        for i in range(P)
    ]
    block = ctx.enter_context(nc.Block(no_gpsimd_drain=True))

    def slab_col(i, buf):
        return (geo["slab_base"][i] + buf) * NC

    @block.gpsimd
    def _(gpsimd):
        for _, _, kind, i, b, fw in geo["items"]:
            if fw is not None:
                gpsimd.wait_ge(mm_sem, fw)
            if kind == "slab0":
                # batched block-0 slabs for the nbuf=4 positions: SBUF
                # buffer-0 columns are a uniform 4*NC stride apart.
                dst = slabs_sb[:, : nb0 * 4 * NC].rearrange(
                    "p (i n) -> p i n", n=4 * NC
                )[:, :, :NC]
                gpsimd.dma_start(dst, slab0_d[:]).then_inc(b0_sem, 16)
            elif kind == "slab":
                nb = geo["nbuf"][i]
                col = slab_col(i, b % nb)
                gpsimd.dma_start(
                    slabs_sb[:, col : col + NC], slab_d[geo["slot_of"][(i, b)]]
                ).then_inc(sl_sems[i][b % nb], 16)
            else:
                y = i
                base = (y % NWRING) * P * 128
                gpsimd.dma_start(
                    wring[:, base : base + P * 128], wts_d[y]
                ).then_inc(wt_sems[y % NWRING], 16)

    @block.tensor
    def _(tensor):
        # Warm the PE HAM clock gate during the DMA prologue with junk
        # matmuls (quiet SBUF regions; bank 7 is cleared by y=7's start).
        wq = (NWRING - 1) * P * 128
        sq = SLABCOLS - NC
        for _ in range(48):
            tensor.matmul(
                out=ps[NBANK - 1][:, :128],
                lhsT=wring[:, wq : wq + 128],
                rhs=slabs_sb[:, sq : sq + 128],
                start=True,
                stop=True,
            )
        for y in range(NY):
            if y >= NBANK:
                tensor.wait_ge(cp_sem, y - NBANK + 1)
            tensor.wait_ge(wt_sems[y % NWRING], 16 * (y // NWRING + 1))
            wbase = (y % NWRING) * P * 128
            for i, g in enumerate(profile):
                nb = geo["nbuf"][i]
                b = y // g
                if y % g == 0:
                    if i in geo["batch0"] and b == 0:
                        tensor.wait_ge(b0_sem, 16)
                    elif i in geo["batch0"] and b % nb == 0:
                        tensor.wait_ge(sl_sems[i][0], 16 * (b // nb))
                    else:
                        tensor.wait_ge(sl_sems[i][b % nb], 16 * (b // nb + 1))
                col = slab_col(i, b % nb)
                mm = tensor.matmul(
                    out=ps[y % NBANK][:],
                    lhsT=wring[:, wbase + i * 128 : wbase + (i + 1) * 128],
                    rhs=slabs_sb[:, col : col + NC],
                    start=(i == 0),
                    stop=(i == P - 1),
                )
            mm.then_inc(mm_sem, 1)

    @block.scalar
    def _(scalar):
        for y in range(NY):
            scalar.wait_ge(mm_sem, y + 1)
            if y >= NOBUF:
                scalar.wait_ge(dump_sem, 16 * (y - NOBUF + 1))
            col = (y % NOBUF) * NC
            scalar.copy(obuf[:, col : col + NC], ps[y % NBANK][:]).then_inc(cp_sem, 1)

    @block.sync
    def _(sync):
        for y in range(NY):
            sync.wait_ge(cp_sem, y + 1)
            col = (y % NOBUF) * NC
            sync.dma_start(out_d[y], obuf[:, col : col + NC]).then_inc(dump_sem, 16)

    _cache["nc"] = nc
    return nc


def _install_ntff_hook():
    """Provide the antenv.axon_hooks shim the image lacks, wiring the
    ctypes NTFF profiler from trn_agent_boot."""
    import sys
    import types

    if "antenv.axon_hooks" in sys.modules:
        return
    import antenv
    from trn_agent_boot.trn_boot import _ntff_profile_via_ctypes

    mod = types.ModuleType("antenv.axon_hooks")
    hook = _ntff_profile_via_ctypes("/opt/axon/libaxon_pjrt.so")
    mod.get_axon_ntff_profile_hook = lambda: hook
    mod.set_axon_ntff_profile_hook = lambda h: None
    sys.modules["antenv.axon_hooks"] = mod
    antenv.axon_hooks = mod


def hw_exec_time_ns(trace_cores=None):
    """Re-run the last kernel() invocation with tracing; return max core ns."""
    _install_ntff_hook()
    nc = _cache["nc"]
    res = run_bass_kernel_spmd(
        nc,
        _cache["in_maps"],
        core_ids=list(range(NCORES)),
        trace=True,
        trace_cores=trace_cores,
    )
    _cache["trace"] = res
    return res.exec_time_ns


def kernel(accumulator, out_H=128, out_W=128, numangle=180, numrho=184):
    accumulator = np.asarray(accumulator, np.float32)
    assert accumulator.shape == (N, C, A, R), accumulator.shape
    assert int(out_H) == H and int(out_W) == W
    assert int(numangle) == A and int(numrho) == R

    geo = _geometry()
    wts, slab_meta = _host_tables()
    nc = _build_nc()

    # acc_t[k, rho, nc] bf16 - slab source.
    acc_t = np.ascontiguousarray(
        accumulator.reshape(NC, A, R).transpose(1, 2, 0)
    ).astype(BF16)

    in_maps = []
    for c in range(NCORES):
        slabs = np.zeros((geo["nslot"], 128, NC), BF16)
        for slot, entries in enumerate(slab_meta[c]):
            for k, o, width, base in entries:
                slabs[slot, base : base + width] = acc_t[k, o : o + width]
        slab0 = np.ascontiguousarray(
            slabs[[geo["slot_of"][(i, 0)] for i in geo["batch0"]]]
            .transpose(1, 0, 2)
        )
        in_maps.append({"wts": wts[c], "slabs": slabs, "slab0": slab0})
    _cache["in_maps"] = in_maps
    res = run_bass_kernel_spmd(nc, in_maps, core_ids=list(range(NCORES)))

    # Unshard: sum the 8 per-core partials.  out[y, x, nc]
    total = np.zeros((NY, 128, NC), np.float64)
    for c in range(NCORES):
        total += res.results[c]["out"]
    return (
        total.transpose(2, 0, 1).reshape(N, C, H, W).astype(np.float32)
    )
